# revision 44
# baseline (speedup 1.0000x reference)
"""CenterLoss kernel for Trainium2 (8 NeuronCores, SPMD data-parallel over batch).

loss = mean_i ||features[i] - centers[labels[i]]||^2

Key algebraic fact: the reference builds the full [N, C] distance matrix but
only reads the label-selected entry per row, so the loss only needs a gather
of each row's center plus elementwise work:

    loss = (1/N) * sum_i ||f_i - c_{y_i}||^2

Per-core plan (N=8192 sharded over 8 cores -> 1024 rows/core):
  - 8 indirect DMAs (SWDGE InstDMACopy with a dynamic AP -- avoids the ~11us
    GPSIMD ucode library load that InstDMAGatherAnt would pay) gather the
    label-selected center rows from DRAM, 128 rows per instruction (the
    indirect path only supports a [128,1] offset AP with a 2D [128,D] dest).
    The serial Q7 descriptor generation (~10ns/row) is the pacing resource;
    per-tile chunking overlaps it with SDMA data movement and with compute.
  - features shard is passed partition-major ([128, 8*512]) so its DMA reads
    contiguous per partition at line rate.
  - per tile j of 128 rows: DVE subtract, then ACT Square activation with
    fused free-dim accumulation -> acc[:, j] (row-wise squared distances).
  - partition-reduce acc with a tiny matmul against ones (PE), then one ACT
    Copy with scale=1/N and accum_out to finish the scalar, DMA it out.
  - host sums the 8 per-core partials (the scalar "all-reduce"/unshard step).

Built on Bacc: its finalize() runs generate_event_semaphores, which splits
multi-semaphore waits into EventSemaphore instructions (TRN2 instructions
hold at most one wait).

Hardcoded shapes: features [8192, 512] f32, labels [8192] int, centers
[10000, 512] f32. Output: f32 scalar.
"""

import numpy as np

import concourse.bacc as bacc
import concourse.bass as bass
import concourse.mybir as mybir
from concourse.bass_utils import run_bass_kernel_spmd
from concourse.tile import TileContext

N = 8192
D = 512
C = 10000
NCORES = 8
N_LOC = N // NCORES  # 1024 rows per core
P = 128
NTILES = N_LOC // P  # 8 tiles of 128 rows


def build_nc() -> bass.Bass:
    nc = bacc.Bacc(
        dynamic_dma_scratch_size=98304,
        enable_partition_id=False,
        enable_asserts=False,
    )

    # features shard, partition-major: feats[p, j*D + d] = f[j*128 + p, d]
    feats = nc.dram_tensor(
        "features_t", [P, NTILES * D], mybir.dt.float32, kind="ExternalInput"
    )
    centers = nc.dram_tensor("centers", [C, D], mybir.dt.float32, kind="ExternalInput")
    # labels, partition-major int32: labels_t[p, j] = labels[j*128 + p]
    labels = nc.dram_tensor(
        "labels_t", [P, NTILES], mybir.dt.int32, kind="ExternalInput"
    )
    out = nc.dram_tensor("partial", [1, 1], mybir.dt.float32, kind="ExternalOutput")

    with TileContext(nc) as tc:
        with (
            tc.tile_pool(name="sbuf", bufs=1) as pool,
            tc.tile_pool(name="psum", bufs=1, space="PSUM") as psum_pool,
        ):
            lab_tile = pool.tile([P, NTILES], mybir.dt.int32)
            # scalar (ACT) HWDGE: its sequencer reaches this earlier than
            # Sync's, so the gather-gating labels land ~0.5us sooner.
            nc.scalar.dma_start(out=lab_tile[:], in_=labels[:])

            ftile = pool.tile([P, NTILES, D], mybir.dt.float32)
            nc.sync.dma_start(
                out=ftile.rearrange("p j d -> p (j d)")[:], in_=feats[:]
            )

            gats = [
                pool.tile([P, D], mybir.dt.float32, name=f"gat{j}", tag=f"gat{j}")
                for j in range(NTILES)
            ]
            for j in range(NTILES):
                nc.gpsimd.indirect_dma_start(
                    out=gats[j][:],
                    out_offset=None,
                    in_=centers[:],
                    in_offset=bass.IndirectOffsetOnAxis(
                        ap=lab_tile[:, j : j + 1], axis=0
                    ),
                )

            acc = pool.tile([P, NTILES], mybir.dt.float32)
            diffs = [
                pool.tile([P, D], mybir.dt.float32, name=f"diff{j}", tag=f"diff{j}")
                for j in range(NTILES)
            ]
            for j in range(NTILES):
                diff = diffs[j]
                nc.vector.tensor_tensor(
                    out=diff[:],
                    in0=ftile[:, j, :],
                    in1=gats[j][:],
                    op=mybir.AluOpType.subtract,
                )
                nc.scalar.activation(
                    out=diff[:],
                    in_=diff[:],
                    func=mybir.ActivationFunctionType.Square,
                    accum_out=acc[:, j : j + 1],
                )

            # ones carries the 1/N scale; matmul partition-reduces acc, DVE
            # sums the per-tile column and the result DMAs straight out.
            ones = pool.tile([P, 1], mybir.dt.float32)
            nc.vector.memset(ones[:], 1.0 / N)
            ps = psum_pool.tile([1, NTILES], mybir.dt.float32, space="PSUM")
            # partition-reduce cols 0-6 as soon as their accums land (hidden
            # under the gather stream); only the last column's matmul sits on
            # the critical tail.
            nc.tensor.matmul(
                out=ps[:, : NTILES - 1],
                lhsT=ones[:],
                rhs=acc[:, : NTILES - 1],
                start=True,
                stop=True,
            )
            nc.tensor.matmul(
                out=ps[:, NTILES - 1 :],
                lhsT=ones[:],
                rhs=acc[:, NTILES - 1 :],
                start=True,
                stop=True,
            )
            res = pool.tile([1, 1], mybir.dt.float32)
            nc.vector.tensor_reduce(
                out=res[:], in_=ps[:], axis=mybir.AxisListType.X, op=mybir.AluOpType.add
            )
            nc.sync.dma_start(out=out[:], in_=res[:])

    nc.finalize()
    return nc


_NC_CACHE: list = []


def get_nc() -> bass.Bass:
    if not _NC_CACHE:
        _NC_CACHE.append(build_nc())
    return _NC_CACHE[0]


def prepare_in_maps(features, labels, centers):
    features = np.ascontiguousarray(np.asarray(features), dtype=np.float32)
    centers = np.ascontiguousarray(np.asarray(centers), dtype=np.float32)
    labels32 = np.asarray(labels).astype(np.int32)

    in_maps = []
    for c in range(NCORES):
        f = features[c * N_LOC : (c + 1) * N_LOC]  # [1024, 512]
        lab = labels32[c * N_LOC : (c + 1) * N_LOC]  # [1024]
        # partition-major layouts: row j*128+p -> partition p, tile j
        f_t = np.ascontiguousarray(
            f.reshape(NTILES, P, D).transpose(1, 0, 2).reshape(P, NTILES * D)
        )
        lab_t = np.ascontiguousarray(lab.reshape(NTILES, P).T)
        in_maps.append({"features_t": f_t, "centers": centers, "labels_t": lab_t})
    return in_maps


def kernel(features, labels, centers):
    nc = get_nc()
    in_maps = prepare_in_maps(features, labels, centers)
    results = run_bass_kernel_spmd(nc, in_maps, list(range(NCORES))).results
    total = sum(float(r["partial"][0, 0]) for r in results)
    return np.float32(total)


# revision 45
# speedup vs baseline: 1.1973x; 1.1973x over previous
"""CenterLoss kernel for Trainium2 (8 NeuronCores, SPMD data-parallel over batch).

loss = mean_i ||features[i] - centers[labels[i]]||^2

Key algebraic fact: the reference builds the full [N, C] distance matrix but
only reads the label-selected entry per row, so the loss only needs a gather
of each row's center plus elementwise work:

    loss = (1/N) * sum_i ||f_i - c_{y_i}||^2

Per-core plan (N=8192 sharded over 8 cores -> 1024 rows/core):
  - 8 indirect DMAs (SWDGE InstDMACopy with a dynamic AP -- avoids the ~11us
    GPSIMD ucode library load that InstDMAGatherAnt would pay) gather the
    label-selected center rows from DRAM, 128 rows per instruction (the
    indirect path only supports a [128,1] offset AP with a 2D [128,D] dest).
    The serial Q7 descriptor generation (~10ns/row) is the pacing resource;
    per-tile chunking overlaps it with SDMA data movement and with compute.
  - features shard is passed partition-major ([128, 8*512]) so its DMA reads
    contiguous per partition at line rate.
  - per tile j of 128 rows: DVE subtract, then ACT Square activation with
    fused free-dim accumulation -> acc[:, j] (row-wise squared distances).
  - partition-reduce acc with a tiny matmul against ones (PE), then one ACT
    Copy with scale=1/N and accum_out to finish the scalar, DMA it out.
  - host sums the 8 per-core partials (the scalar "all-reduce"/unshard step).

Built on Bacc: its finalize() runs generate_event_semaphores, which splits
multi-semaphore waits into EventSemaphore instructions (TRN2 instructions
hold at most one wait).

Hardcoded shapes: features [8192, 512] f32, labels [8192] int, centers
[10000, 512] f32. Output: f32 scalar.
"""

import numpy as np

import concourse.bacc as bacc
import concourse.bass as bass
import concourse.mybir as mybir
from concourse.bass_utils import run_bass_kernel_spmd
from concourse.tile import TileContext

N = 8192
D = 512
C = 10000
NCORES = 8
N_LOC = N // NCORES  # 1024 rows per core
P = 128
NTILES = N_LOC // P  # 8 tiles of 128 rows


def build_nc() -> bass.Bass:
    nc = bacc.Bacc(
        dynamic_dma_scratch_size=98304,
        enable_partition_id=False,
        enable_asserts=False,
    )

    # features shard, partition-major: feats[p, j*D + d] = f[j*128 + p, d]
    feats = nc.dram_tensor(
        "features_t", [P, NTILES * D], mybir.dt.float32, kind="ExternalInput"
    )
    centers = nc.dram_tensor("centers", [C, D], mybir.dt.float32, kind="ExternalInput")
    # labels, partition-major int32: labels_t[p, j] = labels[j*128 + p]
    labels = nc.dram_tensor(
        "labels_t", [P, NTILES], mybir.dt.int32, kind="ExternalInput"
    )
    out = nc.dram_tensor("partial", [1, 1], mybir.dt.float32, kind="ExternalOutput")

    with TileContext(nc) as tc:
        with (
            tc.tile_pool(name="sbuf", bufs=1) as pool,
            tc.tile_pool(name="psum", bufs=1, space="PSUM") as psum_pool,
        ):
            lab_tile = pool.tile([P, NTILES], mybir.dt.int32)
            nc.sync.dma_start(out=lab_tile[:], in_=labels[:])

            ftile = pool.tile([P, NTILES, D], mybir.dt.float32)
            nc.sync.dma_start(
                out=ftile.rearrange("p j d -> p (j d)")[:], in_=feats[:]
            )

            gats = [
                pool.tile([P, D], mybir.dt.float32, name=f"gat{j}", tag=f"gat{j}")
                for j in range(NTILES)
            ]
            for j in range(NTILES):
                nc.gpsimd.indirect_dma_start(
                    out=gats[j][:],
                    out_offset=None,
                    in_=centers[:],
                    in_offset=bass.IndirectOffsetOnAxis(
                        ap=lab_tile[:, j : j + 1], axis=0
                    ),
                )

            acc = pool.tile([P, NTILES], mybir.dt.float32)
            diffs = [
                pool.tile([P, D], mybir.dt.float32, name=f"diff{j}", tag=f"diff{j}")
                for j in range(NTILES)
            ]
            for j in range(NTILES):
                diff = diffs[j]
                nc.vector.tensor_tensor(
                    out=diff[:],
                    in0=ftile[:, j, :],
                    in1=gats[j][:],
                    op=mybir.AluOpType.subtract,
                )
                nc.scalar.activation(
                    out=diff[:],
                    in_=diff[:],
                    func=mybir.ActivationFunctionType.Square,
                    accum_out=acc[:, j : j + 1],
                )

            # ones carries the 1/N scale; matmul partition-reduces acc, DVE
            # sums the per-tile column and the result DMAs straight out.
            ones = pool.tile([P, 1], mybir.dt.float32)
            nc.vector.memset(ones[:], 1.0 / N)
            ps = psum_pool.tile([1, NTILES], mybir.dt.float32, space="PSUM")
            nc.tensor.matmul(out=ps[:], lhsT=ones[:], rhs=acc[:], start=True, stop=True)
            res = pool.tile([1, 1], mybir.dt.float32)
            nc.vector.tensor_reduce(
                out=res[:], in_=ps[:], axis=mybir.AxisListType.X, op=mybir.AluOpType.add
            )
            nc.sync.dma_start(out=out[:], in_=res[:])

    nc.finalize()
    return nc


_NC_CACHE: list = []


def get_nc() -> bass.Bass:
    if not _NC_CACHE:
        _NC_CACHE.append(build_nc())
    return _NC_CACHE[0]


def prepare_in_maps(features, labels, centers):
    features = np.ascontiguousarray(np.asarray(features), dtype=np.float32)
    centers = np.ascontiguousarray(np.asarray(centers), dtype=np.float32)
    labels32 = np.asarray(labels).astype(np.int32)

    in_maps = []
    for c in range(NCORES):
        f = features[c * N_LOC : (c + 1) * N_LOC]  # [1024, 512]
        lab = labels32[c * N_LOC : (c + 1) * N_LOC]  # [1024]
        # partition-major layouts: row j*128+p -> partition p, tile j
        f_t = np.ascontiguousarray(
            f.reshape(NTILES, P, D).transpose(1, 0, 2).reshape(P, NTILES * D)
        )
        lab_t = np.ascontiguousarray(lab.reshape(NTILES, P).T)
        in_maps.append({"features_t": f_t, "centers": centers, "labels_t": lab_t})
    return in_maps


def kernel(features, labels, centers):
    nc = get_nc()
    in_maps = prepare_in_maps(features, labels, centers)
    results = run_bass_kernel_spmd(nc, in_maps, list(range(NCORES))).results
    total = sum(float(r["partial"][0, 0]) for r in results)
    return np.float32(total)
